# revision 34
# baseline (speedup 1.0000x reference)
"""Bass/Tile TRN2 kernel for nn_AttentionLayer (B=2, S=2048, D=1024, H=16).

Sharding: 8 cores = 2 (batch) x 4 (head groups of 4 heads each).
Each core computes Q/K/V projections for its 256 output columns and
full attention for its 4 heads; host reassembles and normalizes.

Device-side design (v3):
  - Host pre-transposes q/k/v to x^T [D, S] and casts x / W to bf16.
  - Q^T, K^T produced head-transposed [e, s]; V natural [s, e] with an
    all-ones column per head (denominator rides PV; set once by memset,
    V bias is applied on the host after normalization).
  - scores^T per head-pair step: two [128, 1024] PSUM tiles (h0, h1),
    drawn from a 3-slot PSUM ring shared with projection tiles
    (6 banks) + 2 banks for PV.  GPSIMD cannot touch PSUM, so exp runs
    h0 -> ScalarE (exact, scale fused) and h1 -> DVE (one-op
    Schraudolph fast-exp: int16(s*A+B) bitcast bf16).
  - QK is emitted h-major with a 4-matmul PV chunk between the pairs,
    which hides the st-ring recycle latency (exp of cpl N's h0 frees
    the slot cpl N+1's h1 needs just in time).
  - PV flipped: E chunks stationary, V' moving; out[sq 128, 65] costs
    65 rows/matmul and lands natural [sq, e].  One PSUM accumulation
    group per [128, 4, 65] bank tile; PV trails QK/exp by PIPE_DEPTH
    steps through a cross-piece software pipeline.
  - Projections are split into ~1us groups, emitted through need()
    (legality) + a paced filler queue.  Q/K bias-copies ride ScalarE
    (activation Identity with per-partition bias), V copies DVE.
  - Outputs are stored (p-major, j-minor) per 512-block via the SP
    hardware DGE so each store is one 1040B-run DMA; host unpermutes,
    divides by the ones-row denominator, and adds the V bias.
"""
import os
import sys

sys.path.insert(0, "/opt/trn_rl_repo")

import numpy as np
import ml_dtypes

BF16NP = ml_dtypes.bfloat16

import concourse.bacc as bacc
import concourse.mybir as mybir
from concourse.tile import TileContext
from concourse.bass_utils import run_bass_kernel_spmd

F32 = mybir.dt.float32
BF16 = mybir.dt.bfloat16
I16 = mybir.dt.int16
AF = mybir.ActivationFunctionType
ALU = mybir.AluOpType

B, S, D, H = 2, 2048, 1024, 16
HD = D // H            # 64
NCORES = 8
HPC = 4                # heads per core
E = HPC * HD           # 256 output cols per core
EV = HPC * (HD + 1)    # 260: V' with ones column per head
DCH = D // 128         # 8 d chunks
SKC = S // 128         # 16 sk chunks
SCALE = 1.0 / np.sqrt(HD)
# Schraudolph fast-exp constants (score scale folded into the multiplier)
FE_A = float(2 ** 23 / np.log(2)) * float(SCALE) / 65536.0
FE_B = float(127 * 2 ** 23 - 486411) / 65536.0
# exp engine per h-pair: A=ScalarE exact, D=DVE fast-exp
FE_PAT = os.environ.get("FE_PAT", "AD")
FE_PAT2 = os.environ.get("FE_PAT2", FE_PAT)     # odd cpls
FE_PAT_LAST = os.environ.get("FE_PAT_LAST", "AD")
BIAS_ENG = os.environ.get("BIAS_ENG", "A")      # Q/K bias copies
VCOPY_ENG = os.environ.get("VCOPY_ENG", "D")    # V copies

# piece order / pacing knobs (searched via sim)
_C04, _C47, _C07 = list(range(0, 4)), list(range(4, 8)), list(range(0, 8))
_C8F, _C0F = list(range(8, 16)), list(range(0, 16))
_ORDERS2 = {
    "S": [(0, 0, _C04), (1, 0, _C04), (0, 0, _C47), (1, 0, _C47),
          (0, 1, _C07), (1, 1, _C07),
          (0, 0, _C8F), (1, 0, _C8F), (0, 1, _C8F), (1, 1, _C8F),
          (0, 2, _C07), (1, 2, _C07), (0, 2, _C8F), (1, 2, _C8F),
          (0, 3, _C07), (1, 3, _C07), (0, 3, _C8F), (1, 3, _C8F)],
    "F": [(0, 0, _C0F), (1, 0, _C0F), (0, 1, _C0F), (1, 1, _C0F),
          (0, 2, _C0F), (1, 2, _C0F), (0, 3, _C0F), (1, 3, _C0F)],
    "G": [(0, 0, _C04), (1, 0, _C04), (0, 0, _C47), (1, 0, _C47),
          (0, 1, _C0F), (1, 1, _C0F), (0, 0, _C8F), (1, 0, _C8F),
          (0, 2, _C0F), (1, 2, _C0F), (0, 3, _C0F), (1, 3, _C0F)],
}
PORDER2 = os.environ.get("PORDER2", "S")
PIECE_ORDER2 = _ORDERS2[PORDER2]
LAST_KEY = tuple(PIECE_ORDER2[-1][:2])
PACE_NUM = int(os.environ.get("PACE_NUM", "1"))
PACE_DEN = int(os.environ.get("PACE_DEN", "1"))
PIPE_DEPTH = int(os.environ.get("PIPE_DEPTH", "3"))
PACE_START = int(os.environ.get("PACE_START", "0"))
PS_BUFS = int(os.environ.get("PS_BUFS", "3"))
E_BUFS = int(os.environ.get("E_BUFS", "10"))
PVA = int(os.environ.get("PVA", "4"))   # PV matmuls emitted mid-step


def build_kernel(repeat: int = 1, loop_n: int = 1):
    nc = bacc.Bacc()
    xqT = nc.dram_tensor("xqT", [D, S], BF16, kind="ExternalInput")
    xkT = nc.dram_tensor("xkT", [D, S], BF16, kind="ExternalInput")
    xvT = nc.dram_tensor("xvT", [D, S], BF16, kind="ExternalInput")
    wq = nc.dram_tensor("wq", [D, E], BF16, kind="ExternalInput")
    wk = nc.dram_tensor("wk", [D, E], BF16, kind="ExternalInput")
    wv = nc.dram_tensor("wv", [D, E], BF16, kind="ExternalInput")
    bq = nc.dram_tensor("bq", [128, 2], F32, kind="ExternalInput")
    bk = nc.dram_tensor("bk", [128, 2], F32, kind="ExternalInput")
    # per-head-contiguous output: [head, S(permuted p-major per 512-block), 65]
    out = nc.dram_tensor("out", [HPC, S, HD + 1], BF16,
                         kind="ExternalOutput")

    with TileContext(nc) as tc:
        with tc.tile_pool(name="wsb", bufs=1) as wsb, \
             tc.tile_pool(name="xsb", bufs=12) as xsb, \
             tc.tile_pool(name="qkv", bufs=1) as qkv, \
             tc.tile_pool(name="esb", bufs=E_BUFS) as esb, \
             tc.tile_pool(name="osb", bufs=4) as osb, \
             tc.tile_pool(name="psp", bufs=PS_BUFS, space="PSUM") as psp, \
             tc.tile_pool(name="pvp", bufs=2, space="PSUM") as pvp:

            # ---- weights / constants; wk first (feeds the first matmuls) ----
            wk_t = wsb.tile([128, DCH, E], BF16)
            wkr = wk.rearrange("(c p) e -> p c e", p=128)
            nc.sync.dma_start(wk_t[:, 0:4], wkr[:, 0:4])
            # touch Exp early so the ACT table load happens during the DMAs;
            # input is a memset tile so it doesn't wait on any DMA
            warm = wsb.tile([128, 1], F32)
            nc.gpsimd.memset(warm[:], 0.0)
            warm2 = wsb.tile([128, 1], F32)
            nc.scalar.activation(warm2[:], warm[:], AF.Exp)
            # PE p-state warmup: dummy matmuls on memset tiles while the
            # first DMAs are in flight, so the 0.65->2.4GHz clock ramp
            # completes before any real matmul issues.  Sized to end just
            # before the first projection's inputs land (~4.2us).
            NWARM = int(os.environ.get("NWARM", "2"))
            if NWARM:
                dmw = wsb.tile([128, 128], BF16, name="dmw")
                nc.gpsimd.memset(dmw[:], 0.0)
                dmx = wsb.tile([128, 512], BF16, name="dmx")
                nc.gpsimd.memset(dmx[:], 0.0)
                dps = psp.tile([128, 512], F32, tag="ps", name="dps")
                for _ in range(NWARM):
                    nc.tensor.matmul(dps[:], dmw[:], dmx[:],
                                     start=True, stop=True)
                dsink = wsb.tile([128, 1], F32, name="dsink")
                nc.vector.tensor_copy(dsink[:], dps[:, 0:1])

            def load_wq():
                wq_t = wsb.tile([128, DCH, E], BF16, name="wq_t")
                wqr = wq.rearrange("(c p) e -> p c e", p=128)
                nc.sync.dma_start(wq_t[:, 0:4], wqr[:, 0:4])
                nc.sync.dma_start(wq_t[:, 4:8], wqr[:, 4:8])
                bq_t = wsb.tile([128, 2], F32, name="bq_t")
                nc.sync.dma_start(bq_t[:], bq[:])
                return wq_t, bq_t

            def load_wv():
                wv_t = wsb.tile([128, DCH, E], BF16, name="wv_t")
                nc.sync.dma_start(wv_t[:], wv.rearrange("(c p) e -> p c e", p=128))
                return wv_t

            def load_x(src, si, chunked=False, defer_half=False):
                sl = slice(512 * si, 512 * (si + 1))
                x_t = xsb.tile([128, DCH, 512], BF16, tag="x", name=f"x_{si}")
                xr = src[:, sl].rearrange("(c p) s -> p c s", p=128)
                if defer_half:
                    nc.sync.dma_start(x_t[:, :, 0:256], xr[:, :, 0:256])
                    return x_t, lambda: nc.sync.dma_start(
                        x_t[:, :, 256:512], xr[:, :, 256:512])
                if chunked:
                    nc.sync.dma_start(x_t[:, 0:4], xr[:, 0:4])
                    nc.sync.dma_start(x_t[:, 4:8], xr[:, 4:8])
                else:
                    nc.sync.dma_start(x_t[:], xr)
                return x_t

            import contextlib

            def body_scope():
                if loop_n > 1:
                    return tc.For_i(0, loop_n, 1)
                return contextlib.nullcontext()

            for _ in range(repeat):
              with body_scope():
                # persistent per-iteration products
                QT_t = qkv.tile([128, 2, S], BF16, tag="QT", name="QT_t")
                KT_t = qkv.tile([128, 4, S], BF16, tag="KT", name="KT_t")
                V_t = qkv.tile([128, SKC, EV], BF16, tag="V", name="V_t")

                # ---- projection groups (~1-1.7us of PE work each) ----
                def proj_qk_group(x_t, w_t, b_t, o_t, si, et, kt):
                    """One et-group of a Q/K projection for s-block si."""
                    sl = slice(512 * si, 512 * (si + 1))
                    ps = psp.tile([128, 512], F32, tag="ps", name="ps_qk")
                    for c in range(DCH):
                        nc.tensor.matmul(
                            ps[:], w_t[:, c, 128 * et:128 * (et + 1)],
                            x_t[:, c], start=(c == 0), stop=(c == DCH - 1))
                    if kt:
                        parts = [(slice(0, 64), 2 * et),
                                 (slice(64, 128), 2 * et + 1)]
                    else:
                        parts = [(slice(0, 128), et)]
                    for psl, pl in parts:
                        if BIAS_ENG == "A":
                            nc.scalar.add(o_t[psl, pl, sl], ps[psl, :],
                                          b_t[psl, et:et + 1])
                        else:
                            nc.vector.tensor_scalar(
                                out=o_t[psl, pl, sl], in0=ps[psl, :],
                                scalar1=b_t[psl, et:et + 1], scalar2=None,
                                op0=ALU.add)

                def proj_v_group(x_t, si, k):
                    """One sk-chunk (128 rows) of the V projection."""
                    psv = psp.tile([128, 512], F32, tag="ps", name="ps_v")
                    for c in range(DCH):
                        nc.tensor.matmul(
                            psv[:, 0:E], x_t[:, c, 128 * k:128 * (k + 1)],
                            wv_t[:, c], start=(c == 0), stop=(c == DCH - 1))
                    # strided copy that skips each head's ones column
                    dst = V_t[:, 4 * si + k, :].rearrange(
                        "(h e) -> h e", h=HPC)[:, :, 0:HD]
                    src = psv[:, 0:E].rearrange("(h e) -> h e", h=HPC)
                    if VCOPY_ENG == "A":
                        nc.scalar.copy(dst, src)
                    else:
                        nc.vector.tensor_copy(dst, src)

                # zero the unused half of each head's K^T so QK can run at
                # K=128 with the full Q^T pair tile as rhs; set the V ones
                # columns once.  No input deps -> runs during the DMA wait.
                for _h in range(4):
                    _lo, _hi = (64, 128) if _h % 2 == 0 else (0, 64)
                    nc.gpsimd.memset(KT_t[_lo:_hi, _h, :], 0.0)
                nc.gpsimd.memset(
                    V_t.rearrange("k (h e) -> k h e", h=HPC)[:, :, :, HD], 1.0)

                # ---- filler queue machinery ----
                x_state = {}

                def ensure_x(kind, si):
                    key = (kind, si)
                    if key not in x_state:
                        src = {"k": xkT, "q": xqT, "v": xvT}[kind]
                        x_state[key] = load_x(src, si, chunked=(key in (("k", 0), ("q", 0), ("k", 1), ("q", 1), ("k", 2), ("v", 2), ("k", 3), ("q", 2), ("v", 3))))
                    return x_state[key]

                emitted = set()
                state = {"cpl": 0, "fill": 0}
                queue = []

                def emit_group(tag):
                    if tag in emitted:
                        return
                    emitted.add(tag)
                    kind = tag[0]
                    if kind == "K":
                        _, si, et = tag
                        proj_qk_group(ensure_x("k", si), wk_t, bk_t, KT_t,
                                      si, et, kt=True)
                    elif kind == "Q":
                        _, sqt, pr = tag
                        proj_qk_group(ensure_x("q", sqt), wq_t, bq_t, QT_t,
                                      sqt, pr, kt=False)
                    else:
                        _, si, k = tag
                        proj_v_group(ensure_x("v", si), si, k)
                    state["fill"] += 1

                def need(*tags):
                    for t in tags:
                        emit_group(t)

                def pace_fill():
                    state["cpl"] += 1
                    while queue and queue[0] in emitted:
                        queue.pop(0)
                    c_eff = state["cpl"] - PACE_START
                    if (queue and c_eff > 0
                            and state["fill"] * PACE_DEN <= c_eff * PACE_NUM):
                        emit_group(queue.pop(0))

                ovs = {}
                # cross-piece PV pipeline: entries [emit_part_fn, stage, fin]
                gq = []

                def gq_step(force=False, depth=PIPE_DEPTH):
                    """Emit the next PV chunk if the pipeline lag is exceeded."""
                    if not gq:
                        return
                    if not force and len(gq) <= depth:
                        return
                    ent = gq[0]
                    ent[0](ent[1])
                    ent[1] += 1
                    if ent[1] == 2:
                        gq.pop(0)
                        if ent[2] is not None:
                            ent[2]()

                def exp_full(st, eng):
                    """exp of one [128,1024] score tile on ACT or DVE."""
                    e_t = esb.tile([128, 1024], BF16, tag="e", name="e_t")
                    if eng == "A":
                        nc.scalar.activation(e_t[:], st[:], AF.Exp,
                                             scale=float(SCALE))
                    else:
                        ti = e_t.bitcast(I16)
                        nc.vector.tensor_scalar(
                            out=ti[:], in0=st[:], scalar1=FE_A,
                            scalar2=FE_B, op0=ALU.mult, op1=ALU.add)
                    return e_t

                def attention_piece(pr, sqt, cklist, first, final,
                                    last=False, pidx=0):
                    sq0 = 512 * sqt
                    ncpl = len(cklist) // 2
                    need(("Q", sqt, pr))
                    pvh = [pvp.tile([128, 4, 80], F32, tag="pv", bufs=2,
                                    name=f"pv{h}") for h in range(2)]

                    def emit_pv_part(ets, cks, cpl, part):
                        """part 0: h0/q0; part 1: rest; parts 10/11/12 stage
                        the final entry as h0-q1 / h1-q0+q1 for tail overlap."""
                        if part in (0, 10):
                            for ck in cks:
                                need(("V", ck // 4, ck % 4))
                        plan = {0: [(0, 0)], 1: [(0, 1), (1, 0), (1, 1)],
                                10: [(0, 0)], 11: [(0, 1)],
                                12: [(1, 0), (1, 1)]}[part]
                        for h, q in plan:
                            hh = 2 * pr + h
                            e_t = ets[h]
                            for j in range(4):
                                nc.tensor.matmul(
                                    pvh[h][:, j, 0:65],
                                    e_t[:, 512 * q + 128 * j:
                                        512 * q + 128 * (j + 1)],
                                    V_t[:, cks[q], 65 * hh:65 * hh + 65],
                                    start=(cpl == 0 and q == 0 and j == 0),
                                    stop=(cpl == ncpl - 1 and q == 1
                                          and j == 3))

                    def finalize(hs=(0, 1)):
                        for h in hs:
                            hh = 2 * pr + h
                            # stored (p-major, j-minor) within the 512-block
                            # so the DMA sees 1040B contiguous runs; host
                            # unpermutes (s = j*128 + p).
                            dst = out[hh, sq0:sq0 + 512, :]
                            dst = dst.rearrange("(p j) e -> p j e", p=128)
                            if first and final:
                                ov = osb.tile([128, 4, 65], F32, tag="ov",
                                              bufs=6, name=f"ov{pr}{sqt}{h}")
                                nc.vector.tensor_copy(ov[:],
                                                      pvh[h][:, :, 0:65])
                                nc.sync.dma_start(dst, ov[:])
                            elif first:
                                ov = osb.tile([128, 4, 65], F32, tag="ov",
                                              bufs=6, name=f"ov{pr}{sqt}{h}")
                                ovs[(pr, sqt, h)] = ov
                                nc.vector.tensor_copy(ov[:],
                                                      pvh[h][:, :, 0:65])
                            elif not final:
                                ov = ovs[(pr, sqt, h)]
                                nc.vector.tensor_tensor(
                                    out=ov[:], in0=ov[:],
                                    in1=pvh[h][:, :, 0:65], op=ALU.add)
                            else:
                                ov = ovs.pop((pr, sqt, h))
                                ovb = osb.tile([128, 4, 65], BF16, tag="ov",
                                               bufs=14, name=f"ovb{pr}{sqt}{h}")
                                nc.vector.tensor_tensor(
                                    out=ovb[:], in0=ov[:],
                                    in1=pvh[h][:, :, 0:65], op=ALU.add)
                                nc.sync.dma_start(dst, ovb[:])

                    depth = 1 if last else PIPE_DEPTH
                    for cpl in range(ncpl):
                        cks = (cklist[2 * cpl], cklist[2 * cpl + 1])
                        for ck in cks:
                            need(("K", ck // 4, pr))
                        if last and cpl == ncpl - 1:
                            pat = FE_PAT_LAST
                        elif cpl % 2 == 0:
                            pat = FE_PAT
                        else:
                            pat = FE_PAT2
                        ets = []
                        for h in range(2):
                            hh = 2 * pr + h
                            st = psp.tile([128, 1024], F32, tag="ps",
                                          name="st")
                            for q in range(2):
                                nc.tensor.matmul(
                                    st[:, 512 * q:512 * (q + 1)],
                                    KT_t[:, hh,
                                         128 * cks[q]:128 * (cks[q] + 1)],
                                    QT_t[:, pr, sq0:sq0 + 512],
                                    start=True, stop=True)
                            ets.append(exp_full(st, pat[h]))
                            if h == 0:
                                # PV chunk between the h-pairs hides the
                                # st-ring recycle latency
                                gq_step(depth=depth)
                        pace_fill()
                        gq.append([
                            lambda p, e=ets, c=cks, pp=cpl:
                                emit_pv_part(e, c, pp, p),
                            0,
                            finalize if cpl == ncpl - 1 else None])
                        gq_step(depth=depth)
                    if last:
                        # staged drain of the final entry: h0's copy+store
                        # chain starts while h1's last PV matmuls still run
                        while len(gq) > 1:
                            gq_step(force=True)
                        fn, stage, fin = gq.pop(0)
                        if stage == 0:
                            fn(10)
                        fn(11)
                        fin(hs=(0,))
                        fn(12)
                        fin(hs=(1,))

                # ---- bootstrap: si=0/1 projections direct; DMA order is
                # wk[0:4] -> xk0[0:4] -> wk[4:8] -> xk0[4:8] -> bk so the
                # first projection group's matmuls start as early as possible
                xk0 = xsb.tile([128, DCH, 512], BF16, tag="x", name="x_0")
                xk0r = xkT[:, 0:512].rearrange("(c p) s -> p c s", p=128)
                nc.sync.dma_start(xk0[:, 0:2], xk0r[:, 0:2])
                nc.sync.dma_start(xk0[:, 2:4], xk0r[:, 2:4])
                nc.sync.dma_start(wk_t[:, 4:8], wkr[:, 4:8])
                nc.sync.dma_start(xk0[:, 4:6], xk0r[:, 4:6])
                nc.sync.dma_start(xk0[:, 6:8], xk0r[:, 6:8])
                x_state[("k", 0)] = xk0
                bk_t = wsb.tile([128, 2], F32)
                nc.sync.dma_start(bk_t[:], bk[:])
                need(("K", 0, 0), ("K", 0, 1))
                wq_t, bq_t = load_wq()
                need(("Q", 0, 0), ("Q", 0, 1))
                need(("K", 1, 0), ("K", 1, 1))
                wv_t = load_wv()
                xv0, fin0 = load_x(xvT, 0, defer_half=True)
                xv1, fin1 = load_x(xvT, 1, defer_half=True)
                x_state[("v", 0)] = xv0
                x_state[("v", 1)] = xv1
                need(("V", 0, 0), ("V", 0, 1))
                need(("V", 1, 0), ("V", 1, 1))
                fin0()
                fin1()
                need(("V", 0, 2), ("V", 0, 3))
                need(("V", 1, 2), ("V", 1, 3))
                if PORDER2 == "S":
                    prefetch = (("q", 1), ("k", 2), ("v", 2), ("k", 3),
                                ("v", 3), ("q", 2), ("q", 3))
                else:
                    prefetch = (("k", 2), ("v", 2), ("k", 3), ("v", 3),
                                ("q", 1), ("q", 2), ("q", 3))
                for _k, _si in prefetch:
                    ensure_x(_k, _si)
                state["fill"] = 0  # bootstrap groups don't count against pacing

                # everything else rides the paced filler queue
                if PORDER2 == "S":
                    queue.extend([
                        ("Q", 1, 0), ("Q", 1, 1),
                        ("K", 2, 0), ("K", 2, 1),
                        ("V", 2, 0), ("V", 2, 1), ("V", 2, 2), ("V", 2, 3),
                        ("K", 3, 0), ("K", 3, 1),
                        ("V", 3, 0), ("V", 3, 1), ("V", 3, 2), ("V", 3, 3),
                        ("Q", 2, 0), ("Q", 2, 1),
                        ("Q", 3, 0), ("Q", 3, 1),
                    ])
                elif PORDER2 == "F4":
                    queue.extend([
                        ("Q", 1, 0), ("Q", 1, 1),
                        ("K", 2, 0), ("K", 3, 0), ("K", 2, 1), ("K", 3, 1),
                        ("V", 2, 0), ("V", 2, 1), ("V", 2, 2), ("V", 2, 3),
                        ("V", 3, 0), ("V", 3, 1), ("V", 3, 2), ("V", 3, 3),
                        ("Q", 2, 0), ("Q", 2, 1),
                        ("Q", 3, 0), ("Q", 3, 1),
                    ])
                else:
                    queue.extend([
                        ("K", 2, 0), ("K", 3, 0), ("K", 2, 1), ("K", 3, 1),
                        ("V", 2, 0), ("V", 2, 1), ("V", 2, 2), ("V", 2, 3),
                        ("V", 3, 0), ("V", 3, 1), ("V", 3, 2), ("V", 3, 3),
                        ("Q", 1, 0), ("Q", 1, 1),
                        ("Q", 2, 0), ("Q", 2, 1),
                        ("Q", 3, 0), ("Q", 3, 1),
                    ])

                seen = {}
                total = {}
                for (pr, sqt, cklist) in PIECE_ORDER2:
                    total[(pr, sqt)] = total.get((pr, sqt), 0) + len(cklist)
                for i, (pr, sqt, cklist) in enumerate(PIECE_ORDER2):
                    first = (pr, sqt) not in seen
                    seen[(pr, sqt)] = seen.get((pr, sqt), 0) + len(cklist)
                    final = seen[(pr, sqt)] == total[(pr, sqt)]
                    attention_piece(pr, sqt, cklist, first, final,
                                    last=(i == len(PIECE_ORDER2) - 1),
                                    pidx=i)
    nc.compile()
    return nc


_NC_CACHE = {}


def _get_nc(repeat: int = 1, loop_n: int = 1):
    key = (repeat, loop_n)
    if key not in _NC_CACHE:
        _NC_CACHE[key] = build_kernel(repeat, loop_n)
    return _NC_CACHE[key]


def _shard_inputs(q, k, v, Wq, bq, Wk, bk, Wv, bv):
    """Build the 8 per-core input maps (host-side marshaling)."""
    xT = {}
    for b in range(B):
        xT[("q", b)] = np.ascontiguousarray(np.asarray(q)[b].T).astype(BF16NP)
        xT[("k", b)] = np.ascontiguousarray(np.asarray(k)[b].T).astype(BF16NP)
        xT[("v", b)] = np.ascontiguousarray(np.asarray(v)[b].T).astype(BF16NP)
    Wq, Wk, Wv = (np.asarray(a, np.float32) for a in (Wq, Wk, Wv))
    bq, bk = (np.asarray(a, np.float32) for a in (bq, bk))
    in_maps = []
    for c in range(NCORES):
        b, g = divmod(c, HPC)
        sl = slice(E * g, E * (g + 1))
        in_maps.append({
            "xqT": xT[("q", b)], "xkT": xT[("k", b)], "xvT": xT[("v", b)],
            "wq": np.ascontiguousarray(Wq[:, sl]).astype(BF16NP),
            "wk": np.ascontiguousarray(Wk[:, sl]).astype(BF16NP),
            "wv": np.ascontiguousarray(Wv[:, sl]).astype(BF16NP),
            "bq": np.ascontiguousarray(bq[sl].reshape(2, 128).T),
            "bk": np.ascontiguousarray(bk[sl].reshape(2, 128).T),
        })
    return in_maps


def kernel(q, k, v, Wq, bq, Wk, bk, Wv, bv):
    nc = _get_nc()
    in_maps = _shard_inputs(q, k, v, Wq, bq, Wk, bk, Wv, bv)
    res = run_bass_kernel_spmd(nc, in_maps, core_ids=list(range(NCORES)))
    bv = np.asarray(bv, np.float32)
    outp = np.empty((B, S, D), np.float32)
    for c in range(NCORES):
        b, g = divmod(c, HPC)
        o = res.results[c]["out"]  # [4, S(permuted), 65]: out cols + denom
        for h in range(HPC):
            # device stored 512-blocks in (p-major, j-minor) row order;
            # true s = block*512 + j*128 + p, stored row = block*512 + p*4 + j
            blk = o[h].astype(np.float32)
            blk = blk.reshape(4, 128, 4, 65).transpose(0, 2, 1, 3)
            blk = blk.reshape(S, 65)
            c0 = E * g + HD * h
            outp[b, :, c0:c0 + HD] = (blk[:, :HD] / blk[:, HD:HD + 1]
                                      + bv[c0:c0 + HD])
    return outp


# revision 49
# speedup vs baseline: 1.0180x; 1.0180x over previous
"""Bass/Tile TRN2 kernel for nn_AttentionLayer (B=2, S=2048, D=1024, H=16).

Sharding: 8 cores = 2 (batch) x 4 (head groups of 4 heads each).
Each core computes Q/K/V projections for its 256 output columns and
full attention for its 4 heads; host reassembles and normalizes.

Device-side design (v3):
  - Host pre-transposes q/k/v to x^T [D, S] and casts x / W to bf16.
  - Q^T, K^T produced head-transposed [e, s]; V natural [s, e] with an
    all-ones column per head (denominator rides PV; set once by memset,
    V bias is applied on the host after normalization).
  - scores^T per head-pair step: two [128, 1024] PSUM tiles (h0, h1),
    drawn from a 3-slot PSUM ring shared with projection tiles
    (6 banks) + 2 banks for PV.  GPSIMD cannot touch PSUM, so exp runs
    h0 -> ScalarE (exact, scale fused) and h1 -> DVE (one-op
    Schraudolph fast-exp: int16(s*A+B) bitcast bf16).
  - QK is emitted h-major with a 4-matmul PV chunk between the pairs,
    which hides the st-ring recycle latency (exp of cpl N's h0 frees
    the slot cpl N+1's h1 needs just in time).
  - PV flipped: E chunks stationary, V' moving; out[sq 128, 65] costs
    65 rows/matmul and lands natural [sq, e].  One PSUM accumulation
    group per [128, 4, 65] bank tile; PV trails QK/exp by PIPE_DEPTH
    steps through a cross-piece software pipeline.
  - Projections are split into ~1us groups, emitted through need()
    (legality) + a paced filler queue.  Q/K bias-copies ride ScalarE
    (activation Identity with per-partition bias), V copies DVE.
  - Outputs are stored (p-major, j-minor) per 512-block via the SP
    hardware DGE so each store is one 1040B-run DMA; host unpermutes,
    divides by the ones-row denominator, and adds the V bias.
"""
import os
import sys

sys.path.insert(0, "/opt/trn_rl_repo")

import numpy as np
import ml_dtypes

BF16NP = ml_dtypes.bfloat16

import concourse.bacc as bacc
import concourse.mybir as mybir
from concourse.tile import TileContext
from concourse.bass_utils import run_bass_kernel_spmd

F32 = mybir.dt.float32
BF16 = mybir.dt.bfloat16
I16 = mybir.dt.int16
AF = mybir.ActivationFunctionType
ALU = mybir.AluOpType

B, S, D, H = 2, 2048, 1024, 16
HD = D // H            # 64
NCORES = 8
HPC = 4                # heads per core
E = HPC * HD           # 256 output cols per core
EV = HPC * (HD + 1)    # 260: V' with ones column per head
DCH = D // 128         # 8 d chunks
SKC = S // 128         # 16 sk chunks
SCALE = 1.0 / np.sqrt(HD)
# Schraudolph fast-exp constants (score scale folded into the multiplier)
FE_A = float(2 ** 23 / np.log(2)) * float(SCALE) / 65536.0
FE_B = float(127 * 2 ** 23 - 486411) / 65536.0
# exp engine per h-pair: A=ScalarE exact, D=DVE fast-exp
FE_PAT = os.environ.get("FE_PAT", "AD")
FE_PAT2 = os.environ.get("FE_PAT2", FE_PAT)     # odd cpls
FE_PAT_LAST = os.environ.get("FE_PAT_LAST", "AD")
BIAS_ENG = os.environ.get("BIAS_ENG", "A")      # Q/K bias copies
VCOPY_ENG = os.environ.get("VCOPY_ENG", "D")    # V copies

# piece order / pacing knobs (searched via sim)
_C04, _C47, _C07 = list(range(0, 4)), list(range(4, 8)), list(range(0, 8))
_C8F, _C0F = list(range(8, 16)), list(range(0, 16))
_ORDERS2 = {
    "S": [(0, 0, _C04), (1, 0, _C04), (0, 0, _C47), (1, 0, _C47),
          (0, 1, _C07), (1, 1, _C07),
          (0, 0, _C8F), (1, 0, _C8F), (0, 1, _C8F), (1, 1, _C8F),
          (0, 2, _C07), (1, 2, _C07), (0, 2, _C8F), (1, 2, _C8F),
          (0, 3, _C07), (1, 3, _C07), (0, 3, _C8F), (1, 3, _C8F)],
    "F": [(0, 0, _C0F), (1, 0, _C0F), (0, 1, _C0F), (1, 1, _C0F),
          (0, 2, _C0F), (1, 2, _C0F), (0, 3, _C0F), (1, 3, _C0F)],
    "G": [(0, 0, _C04), (1, 0, _C04), (0, 0, _C47), (1, 0, _C47),
          (0, 1, _C0F), (1, 1, _C0F), (0, 0, _C8F), (1, 0, _C8F),
          (0, 2, _C0F), (1, 2, _C0F), (0, 3, _C0F), (1, 3, _C0F)],
}
PORDER2 = os.environ.get("PORDER2", "S")
PIECE_ORDER2 = _ORDERS2[PORDER2]
LAST_KEY = tuple(PIECE_ORDER2[-1][:2])
PACE_NUM = int(os.environ.get("PACE_NUM", "1"))
PACE_DEN = int(os.environ.get("PACE_DEN", "1"))
PIPE_DEPTH = int(os.environ.get("PIPE_DEPTH", "3"))
PACE_START = int(os.environ.get("PACE_START", "0"))
PS_BUFS = int(os.environ.get("PS_BUFS", "3"))
E_BUFS = int(os.environ.get("E_BUFS", "10"))
PVA = int(os.environ.get("PVA", "4"))   # PV matmuls emitted mid-step


def build_kernel(repeat: int = 1, loop_n: int = 1):
    nc = bacc.Bacc()
    xqT = nc.dram_tensor("xqT", [D, S], BF16, kind="ExternalInput")
    xkT = nc.dram_tensor("xkT", [D, S], BF16, kind="ExternalInput")
    xvT = nc.dram_tensor("xvT", [D, S], BF16, kind="ExternalInput")
    wq = nc.dram_tensor("wq", [D, E], BF16, kind="ExternalInput")
    wk = nc.dram_tensor("wk", [D, E], BF16, kind="ExternalInput")
    wv = nc.dram_tensor("wv", [D, E], BF16, kind="ExternalInput")
    bq = nc.dram_tensor("bq", [128, 2], F32, kind="ExternalInput")
    bk = nc.dram_tensor("bk", [128, 2], F32, kind="ExternalInput")
    # per-head-contiguous output: [head, S(permuted p-major per 512-block), 65]
    out = nc.dram_tensor("out", [HPC, S, HD + 1], BF16,
                         kind="ExternalOutput")

    with TileContext(nc) as tc:
        with tc.tile_pool(name="wsb", bufs=1) as wsb, \
             tc.tile_pool(name="xsb", bufs=12) as xsb, \
             tc.tile_pool(name="qkv", bufs=1) as qkv, \
             tc.tile_pool(name="esb", bufs=E_BUFS) as esb, \
             tc.tile_pool(name="osb", bufs=4) as osb, \
             tc.tile_pool(name="psp", bufs=PS_BUFS, space="PSUM") as psp, \
             tc.tile_pool(name="pvp", bufs=2, space="PSUM") as pvp:

            # ---- weights / constants; wk first (feeds the first matmuls) ----
            wk_t = wsb.tile([128, DCH, E], BF16)
            wkr = wk.rearrange("(c p) e -> p c e", p=128)
            nc.sync.dma_start(wk_t[:, 0:4], wkr[:, 0:4])
            # touch Exp early so the ACT table load happens during the DMAs;
            # input is a memset tile so it doesn't wait on any DMA
            warm = wsb.tile([128, 1], F32)
            nc.gpsimd.memset(warm[:], 0.0)
            warm2 = wsb.tile([128, 1], F32)
            nc.scalar.activation(warm2[:], warm[:], AF.Exp)
            # PE p-state warmup: dummy matmuls on memset tiles while the
            # first DMAs are in flight, so the 0.65->2.4GHz clock ramp
            # completes before any real matmul issues.  Sized to end just
            # before the first projection's inputs land (~4.2us).
            NWARM = int(os.environ.get("NWARM", "2"))
            if NWARM:
                dmw = wsb.tile([128, 128], BF16, name="dmw")
                nc.gpsimd.memset(dmw[:], 0.0)
                dmx = wsb.tile([128, 512], BF16, name="dmx")
                nc.gpsimd.memset(dmx[:], 0.0)
                dps = psp.tile([128, 512], F32, tag="ps", name="dps")
                for _ in range(NWARM):
                    nc.tensor.matmul(dps[:], dmw[:], dmx[:],
                                     start=True, stop=True)
                dsink = wsb.tile([128, 1], F32, name="dsink")
                nc.vector.tensor_copy(dsink[:], dps[:, 0:1])

            def load_wq():
                wq_t = wsb.tile([128, DCH, E], BF16, name="wq_t")
                wqr = wq.rearrange("(c p) e -> p c e", p=128)
                nc.sync.dma_start(wq_t[:, 0:4], wqr[:, 0:4])
                nc.sync.dma_start(wq_t[:, 4:8], wqr[:, 4:8])
                bq_t = wsb.tile([128, 2], F32, name="bq_t")
                nc.sync.dma_start(bq_t[:], bq[:])
                return wq_t, bq_t

            def load_wv():
                wv_t = wsb.tile([128, DCH, E], BF16, name="wv_t")
                nc.sync.dma_start(wv_t[:], wv.rearrange("(c p) e -> p c e", p=128))
                return wv_t

            def load_x(src, si, chunked=False, defer_half=False):
                sl = slice(512 * si, 512 * (si + 1))
                x_t = xsb.tile([128, DCH, 512], BF16, tag="x", name=f"x_{si}")
                xr = src[:, sl].rearrange("(c p) s -> p c s", p=128)
                if defer_half:
                    nc.sync.dma_start(x_t[:, :, 0:256], xr[:, :, 0:256])
                    return x_t, lambda: nc.sync.dma_start(
                        x_t[:, :, 256:512], xr[:, :, 256:512])
                if chunked:
                    nc.sync.dma_start(x_t[:, 0:4], xr[:, 0:4])
                    nc.sync.dma_start(x_t[:, 4:8], xr[:, 4:8])
                else:
                    nc.sync.dma_start(x_t[:], xr)
                return x_t

            import contextlib

            def body_scope():
                if loop_n > 1:
                    return tc.For_i(0, loop_n, 1)
                return contextlib.nullcontext()

            for _ in range(repeat):
              with body_scope():
                # persistent per-iteration products
                QT_t = qkv.tile([128, 2, S], BF16, tag="QT", name="QT_t")
                KT_t = qkv.tile([128, 2, S], BF16, tag="KT", name="KT_t")
                V_t = qkv.tile([128, SKC, EV], BF16, tag="V", name="V_t")

                # ---- projection groups (~1-1.7us of PE work each) ----
                def proj_qk_group(x_t, w_t, b_t, o_t, si, et, kt):
                    """One et-group of a Q/K projection for s-block si."""
                    sl = slice(512 * si, 512 * (si + 1))
                    ps = psp.tile([128, 512], F32, tag="ps", name="ps_qk")
                    for c in range(DCH):
                        nc.tensor.matmul(
                            ps[:], w_t[:, c, 128 * et:128 * (et + 1)],
                            x_t[:, c], start=(c == 0), stop=(c == DCH - 1))
                    if BIAS_ENG == "A":
                        nc.scalar.add(o_t[:, et, sl], ps[:],
                                      b_t[:, et:et + 1])
                    else:
                        nc.vector.tensor_scalar(
                            out=o_t[:, et, sl], in0=ps[:],
                            scalar1=b_t[:, et:et + 1], scalar2=None,
                            op0=ALU.add)

                def proj_v_group(x_t, si, k):
                    """One sk-chunk (128 rows) of the V projection."""
                    psv = psp.tile([128, 512], F32, tag="ps", name="ps_v")
                    for c in range(DCH):
                        nc.tensor.matmul(
                            psv[:, 0:E], x_t[:, c, 128 * k:128 * (k + 1)],
                            wv_t[:, c], start=(c == 0), stop=(c == DCH - 1))
                    # strided copy that skips each head's ones column
                    dst = V_t[:, 4 * si + k, :].rearrange(
                        "(h e) -> h e", h=HPC)[:, :, 0:HD]
                    src = psv[:, 0:E].rearrange("(h e) -> h e", h=HPC)
                    if VCOPY_ENG == "A":
                        nc.scalar.copy(dst, src)
                    else:
                        nc.vector.tensor_copy(dst, src)

                # set the V ones columns once (no input deps -> runs
                # during the initial DMA wait)
                nc.gpsimd.memset(
                    V_t.rearrange("k (h e) -> k h e", h=HPC)[:, :, :, HD], 1.0)

                # ---- filler queue machinery ----
                x_state = {}

                def ensure_x(kind, si):
                    key = (kind, si)
                    if key not in x_state:
                        src = {"k": xkT, "q": xqT, "v": xvT}[kind]
                        x_state[key] = load_x(src, si, chunked=(key in (("k", 0), ("q", 0), ("k", 1), ("q", 1), ("k", 2), ("v", 2), ("k", 3), ("q", 2), ("v", 3))))
                    return x_state[key]

                emitted = set()
                state = {"cpl": 0, "fill": 0}
                queue = []

                def emit_group(tag):
                    if tag in emitted:
                        return
                    emitted.add(tag)
                    kind = tag[0]
                    if kind == "K":
                        _, si, et = tag
                        proj_qk_group(ensure_x("k", si), wk_t, bk_t, KT_t,
                                      si, et, kt=True)
                    elif kind == "Q":
                        _, sqt, pr = tag
                        proj_qk_group(ensure_x("q", sqt), wq_t, bq_t, QT_t,
                                      sqt, pr, kt=False)
                    else:
                        _, si, k = tag
                        proj_v_group(ensure_x("v", si), si, k)
                    state["fill"] += 1

                def need(*tags):
                    for t in tags:
                        emit_group(t)

                def pace_fill():
                    state["cpl"] += 1
                    while queue and queue[0] in emitted:
                        queue.pop(0)
                    c_eff = state["cpl"] - PACE_START
                    if (queue and c_eff > 0
                            and state["fill"] * PACE_DEN <= c_eff * PACE_NUM):
                        emit_group(queue.pop(0))

                ovs = {}
                # cross-piece PV pipeline: entries [emit_part_fn, stage, fin]
                gq = []

                def gq_step(force=False, depth=PIPE_DEPTH):
                    """Emit the next PV chunk if the pipeline lag is exceeded."""
                    if not gq:
                        return
                    if not force and len(gq) <= depth:
                        return
                    ent = gq[0]
                    ent[0](ent[1])
                    ent[1] += 1
                    if ent[1] == 2:
                        gq.pop(0)
                        if ent[2] is not None:
                            ent[2]()

                def exp_full(st, eng):
                    """exp of one [128,1024] score tile on ACT or DVE."""
                    e_t = esb.tile([128, 1024], BF16, tag="e", name="e_t")
                    if eng == "A":
                        nc.scalar.activation(e_t[:], st[:], AF.Exp,
                                             scale=float(SCALE))
                    else:
                        ti = e_t.bitcast(I16)
                        nc.vector.tensor_scalar(
                            out=ti[:], in0=st[:], scalar1=FE_A,
                            scalar2=FE_B, op0=ALU.mult, op1=ALU.add)
                    return e_t

                def attention_piece(pr, sqt, cklist, first, final,
                                    last=False, pidx=0):
                    sq0 = 512 * sqt
                    ncpl = len(cklist) // 2
                    need(("Q", sqt, pr))
                    pvh = [pvp.tile([128, 4, 80], F32, tag="pv", bufs=2,
                                    name=f"pv{h}") for h in range(2)]

                    def emit_pv_part(ets, cks, cpl, part):
                        """part 0: h0/q0; part 1: rest; parts 10/11/12 stage
                        the final entry as h0-q1 / h1-q0+q1 for tail overlap."""
                        if part in (0, 10):
                            for ck in cks:
                                need(("V", ck // 4, ck % 4))
                        if PVA == 8:
                            p0, p1 = [(0, 0), (0, 1)], [(1, 0), (1, 1)]
                        else:
                            p0, p1 = [(0, 0)], [(0, 1), (1, 0), (1, 1)]
                        plan = {0: p0, 1: p1,
                                10: [(0, 0)], 11: [(0, 1)],
                                12: [(1, 0), (1, 1)]}[part]
                        for h, q in plan:
                            hh = 2 * pr + h
                            e_t = ets[h]
                            for j in range(4):
                                nc.tensor.matmul(
                                    pvh[h][:, j, 0:65],
                                    e_t[:, 512 * q + 128 * j:
                                        512 * q + 128 * (j + 1)],
                                    V_t[:, cks[q], 65 * hh:65 * hh + 65],
                                    start=(cpl == 0 and q == 0 and j == 0),
                                    stop=(cpl == ncpl - 1 and q == 1
                                          and j == 3))

                    def finalize(hs=(0, 1)):
                        for h in hs:
                            hh = 2 * pr + h
                            # stored (p-major, j-minor) within the 512-block
                            # so the DMA sees 1040B contiguous runs; host
                            # unpermutes (s = j*128 + p).
                            dst = out[hh, sq0:sq0 + 512, :]
                            dst = dst.rearrange("(p j) e -> p j e", p=128)
                            if first and final:
                                ov = osb.tile([128, 4, 65], F32, tag="ov",
                                              bufs=6, name=f"ov{pr}{sqt}{h}")
                                nc.vector.tensor_copy(ov[:],
                                                      pvh[h][:, :, 0:65])
                                nc.sync.dma_start(dst, ov[:])
                            elif first:
                                ov = osb.tile([128, 4, 65], F32, tag="ov",
                                              bufs=6, name=f"ov{pr}{sqt}{h}")
                                ovs[(pr, sqt, h)] = ov
                                nc.vector.tensor_copy(ov[:],
                                                      pvh[h][:, :, 0:65])
                            elif not final:
                                ov = ovs[(pr, sqt, h)]
                                nc.vector.tensor_tensor(
                                    out=ov[:], in0=ov[:],
                                    in1=pvh[h][:, :, 0:65], op=ALU.add)
                            else:
                                ov = ovs.pop((pr, sqt, h))
                                ovb = osb.tile([128, 4, 65], BF16, tag="ov",
                                               bufs=14, name=f"ovb{pr}{sqt}{h}")
                                nc.vector.tensor_tensor(
                                    out=ovb[:], in0=ov[:],
                                    in1=pvh[h][:, :, 0:65], op=ALU.add)
                                nc.sync.dma_start(dst, ovb[:])

                    depth = 1 if last else PIPE_DEPTH
                    for cpl in range(ncpl):
                        cks = (cklist[2 * cpl], cklist[2 * cpl + 1])
                        for ck in cks:
                            need(("K", ck // 4, pr))
                        if last and cpl == ncpl - 1:
                            pat = FE_PAT_LAST
                        elif cpl % 2 == 0:
                            pat = FE_PAT
                        else:
                            pat = FE_PAT2
                        ets = []
                        for h in range(2):
                            hh = 2 * pr + h
                            st = psp.tile([128, 1024], F32, tag="ps",
                                          name="st")
                            hsl = slice(64 * h, 64 * h + 64)
                            for q in range(2):
                                nc.tensor.matmul(
                                    st[:, 512 * q:512 * (q + 1)],
                                    KT_t[hsl, pr,
                                         128 * cks[q]:128 * (cks[q] + 1)],
                                    QT_t[hsl, pr, sq0:sq0 + 512],
                                    start=True, stop=True)
                            ets.append(exp_full(st, pat[h]))
                            if h == 0:
                                # PV chunk between the h-pairs hides the
                                # st-ring recycle latency
                                gq_step(depth=depth)
                        pace_fill()
                        gq.append([
                            lambda p, e=ets, c=cks, pp=cpl:
                                emit_pv_part(e, c, pp, p),
                            0,
                            finalize if cpl == ncpl - 1 else None])
                        gq_step(depth=depth)
                    if last:
                        # staged drain of the final entry: h0's copy+store
                        # chain starts while h1's last PV matmuls still run
                        while len(gq) > 1:
                            gq_step(force=True)
                        fn, stage, fin = gq.pop(0)
                        if stage == 0:
                            fn(10)
                        fn(11)
                        fin(hs=(0,))
                        fn(12)
                        fin(hs=(1,))

                # ---- bootstrap: si=0/1 projections direct; DMA order is
                # wk[0:4] -> xk0[0:4] -> wk[4:8] -> xk0[4:8] -> bk so the
                # first projection group's matmuls start as early as possible
                xk0 = xsb.tile([128, DCH, 512], BF16, tag="x", name="x_0")
                xk0r = xkT[:, 0:512].rearrange("(c p) s -> p c s", p=128)
                nc.sync.dma_start(xk0[:, 0:2], xk0r[:, 0:2])
                nc.sync.dma_start(xk0[:, 2:4], xk0r[:, 2:4])
                nc.sync.dma_start(wk_t[:, 4:8], wkr[:, 4:8])
                nc.sync.dma_start(xk0[:, 4:6], xk0r[:, 4:6])
                nc.sync.dma_start(xk0[:, 6:8], xk0r[:, 6:8])
                x_state[("k", 0)] = xk0
                bk_t = wsb.tile([128, 2], F32)
                nc.sync.dma_start(bk_t[:], bk[:])
                need(("K", 0, 0), ("K", 0, 1))
                wq_t, bq_t = load_wq()
                need(("Q", 0, 0), ("Q", 0, 1))
                need(("K", 1, 0), ("K", 1, 1))
                wv_t = load_wv()
                xv0, fin0 = load_x(xvT, 0, defer_half=True)
                xv1, fin1 = load_x(xvT, 1, defer_half=True)
                x_state[("v", 0)] = xv0
                x_state[("v", 1)] = xv1
                need(("V", 0, 0), ("V", 0, 1))
                need(("V", 1, 0), ("V", 1, 1))
                fin0()
                fin1()
                need(("V", 0, 2), ("V", 0, 3))
                need(("V", 1, 2), ("V", 1, 3))
                if PORDER2 == "S":
                    prefetch = (("q", 1), ("k", 2), ("v", 2), ("k", 3),
                                ("v", 3), ("q", 2), ("q", 3))
                else:
                    prefetch = (("k", 2), ("v", 2), ("k", 3), ("v", 3),
                                ("q", 1), ("q", 2), ("q", 3))
                for _k, _si in prefetch:
                    ensure_x(_k, _si)
                state["fill"] = 0  # bootstrap groups don't count against pacing

                # everything else rides the paced filler queue
                if PORDER2 == "S":
                    queue.extend([
                        ("Q", 1, 0), ("Q", 1, 1),
                        ("K", 2, 0), ("K", 2, 1),
                        ("V", 2, 0), ("V", 2, 1), ("V", 2, 2), ("V", 2, 3),
                        ("K", 3, 0), ("K", 3, 1),
                        ("V", 3, 0), ("V", 3, 1), ("V", 3, 2), ("V", 3, 3),
                        ("Q", 2, 0), ("Q", 2, 1),
                        ("Q", 3, 0), ("Q", 3, 1),
                    ])
                elif PORDER2 == "F4":
                    queue.extend([
                        ("Q", 1, 0), ("Q", 1, 1),
                        ("K", 2, 0), ("K", 3, 0), ("K", 2, 1), ("K", 3, 1),
                        ("V", 2, 0), ("V", 2, 1), ("V", 2, 2), ("V", 2, 3),
                        ("V", 3, 0), ("V", 3, 1), ("V", 3, 2), ("V", 3, 3),
                        ("Q", 2, 0), ("Q", 2, 1),
                        ("Q", 3, 0), ("Q", 3, 1),
                    ])
                else:
                    queue.extend([
                        ("K", 2, 0), ("K", 3, 0), ("K", 2, 1), ("K", 3, 1),
                        ("V", 2, 0), ("V", 2, 1), ("V", 2, 2), ("V", 2, 3),
                        ("V", 3, 0), ("V", 3, 1), ("V", 3, 2), ("V", 3, 3),
                        ("Q", 1, 0), ("Q", 1, 1),
                        ("Q", 2, 0), ("Q", 2, 1),
                        ("Q", 3, 0), ("Q", 3, 1),
                    ])

                seen = {}
                total = {}
                for (pr, sqt, cklist) in PIECE_ORDER2:
                    total[(pr, sqt)] = total.get((pr, sqt), 0) + len(cklist)
                for i, (pr, sqt, cklist) in enumerate(PIECE_ORDER2):
                    first = (pr, sqt) not in seen
                    seen[(pr, sqt)] = seen.get((pr, sqt), 0) + len(cklist)
                    final = seen[(pr, sqt)] == total[(pr, sqt)]
                    attention_piece(pr, sqt, cklist, first, final,
                                    last=(i == len(PIECE_ORDER2) - 1),
                                    pidx=i)
    nc.compile()
    return nc


_NC_CACHE = {}


def _get_nc(repeat: int = 1, loop_n: int = 1):
    key = (repeat, loop_n)
    if key not in _NC_CACHE:
        _NC_CACHE[key] = build_kernel(repeat, loop_n)
    return _NC_CACHE[key]


def _shard_inputs(q, k, v, Wq, bq, Wk, bk, Wv, bv):
    """Build the 8 per-core input maps (host-side marshaling)."""
    xT = {}
    for b in range(B):
        xT[("q", b)] = np.ascontiguousarray(np.asarray(q)[b].T).astype(BF16NP)
        xT[("k", b)] = np.ascontiguousarray(np.asarray(k)[b].T).astype(BF16NP)
        xT[("v", b)] = np.ascontiguousarray(np.asarray(v)[b].T).astype(BF16NP)
    Wq, Wk, Wv = (np.asarray(a, np.float32) for a in (Wq, Wk, Wv))
    bq, bk = (np.asarray(a, np.float32) for a in (bq, bk))
    in_maps = []
    for c in range(NCORES):
        b, g = divmod(c, HPC)
        sl = slice(E * g, E * (g + 1))
        in_maps.append({
            "xqT": xT[("q", b)], "xkT": xT[("k", b)], "xvT": xT[("v", b)],
            "wq": np.ascontiguousarray(Wq[:, sl]).astype(BF16NP),
            "wk": np.ascontiguousarray(Wk[:, sl]).astype(BF16NP),
            "wv": np.ascontiguousarray(Wv[:, sl]).astype(BF16NP),
            "bq": np.ascontiguousarray(bq[sl].reshape(2, 128).T),
            "bk": np.ascontiguousarray(bk[sl].reshape(2, 128).T),
        })
    return in_maps


def kernel(q, k, v, Wq, bq, Wk, bk, Wv, bv):
    nc = _get_nc()
    in_maps = _shard_inputs(q, k, v, Wq, bq, Wk, bk, Wv, bv)
    res = run_bass_kernel_spmd(nc, in_maps, core_ids=list(range(NCORES)))
    bv = np.asarray(bv, np.float32)
    outp = np.empty((B, S, D), np.float32)
    for c in range(NCORES):
        b, g = divmod(c, HPC)
        o = res.results[c]["out"]  # [4, S(permuted), 65]: out cols + denom
        for h in range(HPC):
            # device stored 512-blocks in (p-major, j-minor) row order;
            # true s = block*512 + j*128 + p, stored row = block*512 + p*4 + j
            blk = o[h].astype(np.float32)
            blk = blk.reshape(4, 128, 4, 65).transpose(0, 2, 1, 3)
            blk = blk.reshape(S, 65)
            c0 = E * g + HD * h
            outp[b, :, c0:c0 + HD] = (blk[:, :HD] / blk[:, HD:HD + 1]
                                      + bv[c0:c0 + HD])
    return outp


# revision 50
# speedup vs baseline: 1.0184x; 1.0004x over previous
"""Bass/Tile TRN2 kernel for nn_AttentionLayer (B=2, S=2048, D=1024, H=16).

Sharding: 8 cores = 2 (batch) x 4 (head groups of 4 heads each).
Each core computes Q/K/V projections for its 256 output columns and
full attention for its 4 heads; host reassembles and normalizes.

Device-side design (v3):
  - Host pre-transposes q/k/v to x^T [D, S] and casts x / W to bf16.
  - Q^T, K^T produced head-transposed [e, s]; V natural [s, e] with an
    all-ones column per head (denominator rides PV; set once by memset,
    V bias is applied on the host after normalization).
  - scores^T per head-pair step: two [128, 1024] PSUM tiles (h0, h1),
    drawn from a 3-slot PSUM ring shared with projection tiles
    (6 banks) + 2 banks for PV.  GPSIMD cannot touch PSUM, so exp runs
    h0 -> ScalarE (exact, scale fused) and h1 -> DVE (one-op
    Schraudolph fast-exp: int16(s*A+B) bitcast bf16).
  - QK is emitted h-major with a 4-matmul PV chunk between the pairs,
    which hides the st-ring recycle latency (exp of cpl N's h0 frees
    the slot cpl N+1's h1 needs just in time).
  - PV flipped: E chunks stationary, V' moving; out[sq 128, 65] costs
    65 rows/matmul and lands natural [sq, e].  One PSUM accumulation
    group per [128, 4, 65] bank tile; PV trails QK/exp by PIPE_DEPTH
    steps through a cross-piece software pipeline.
  - Projections are split into ~1us groups, emitted through need()
    (legality) + a paced filler queue.  Q/K bias-copies ride ScalarE
    (activation Identity with per-partition bias), V copies DVE.
  - Outputs are stored (p-major, j-minor) per 512-block via the SP
    hardware DGE so each store is one 1040B-run DMA; host unpermutes,
    divides by the ones-row denominator, and adds the V bias.
"""
import os
import sys

sys.path.insert(0, "/opt/trn_rl_repo")

import numpy as np
import ml_dtypes

BF16NP = ml_dtypes.bfloat16

import concourse.bacc as bacc
import concourse.mybir as mybir
from concourse.tile import TileContext
from concourse.bass_utils import run_bass_kernel_spmd

F32 = mybir.dt.float32
BF16 = mybir.dt.bfloat16
I16 = mybir.dt.int16
AF = mybir.ActivationFunctionType
ALU = mybir.AluOpType

B, S, D, H = 2, 2048, 1024, 16
HD = D // H            # 64
NCORES = 8
HPC = 4                # heads per core
E = HPC * HD           # 256 output cols per core
EV = HPC * (HD + 1)    # 260: V' with ones column per head
DCH = D // 128         # 8 d chunks
SKC = S // 128         # 16 sk chunks
SCALE = 1.0 / np.sqrt(HD)
# Schraudolph fast-exp constants (score scale folded into the multiplier)
FE_A = float(2 ** 23 / np.log(2)) * float(SCALE) / 65536.0
FE_B = float(127 * 2 ** 23 - 486411) / 65536.0
# exp engine per h-pair: A=ScalarE exact, D=DVE fast-exp
FE_PAT = os.environ.get("FE_PAT", "AD")
FE_PAT2 = os.environ.get("FE_PAT2", FE_PAT)     # odd cpls
FE_PAT_LAST = os.environ.get("FE_PAT_LAST", "AD")
BIAS_ENG = os.environ.get("BIAS_ENG", "A")      # Q/K bias copies
VCOPY_ENG = os.environ.get("VCOPY_ENG", "D")    # V copies

# piece order / pacing knobs (searched via sim)
_C04, _C47, _C07 = list(range(0, 4)), list(range(4, 8)), list(range(0, 8))
_C8F, _C0F = list(range(8, 16)), list(range(0, 16))
_ORDERS2 = {
    "S": [(0, 0, _C04), (1, 0, _C04), (0, 0, _C47), (1, 0, _C47),
          (0, 1, _C07), (1, 1, _C07),
          (0, 0, _C8F), (1, 0, _C8F), (0, 1, _C8F), (1, 1, _C8F),
          (0, 2, _C07), (1, 2, _C07), (0, 2, _C8F), (1, 2, _C8F),
          (0, 3, _C07), (1, 3, _C07), (0, 3, _C8F), (1, 3, _C8F)],
    "F": [(0, 0, _C0F), (1, 0, _C0F), (0, 1, _C0F), (1, 1, _C0F),
          (0, 2, _C0F), (1, 2, _C0F), (0, 3, _C0F), (1, 3, _C0F)],
    "G": [(0, 0, _C04), (1, 0, _C04), (0, 0, _C47), (1, 0, _C47),
          (0, 1, _C0F), (1, 1, _C0F), (0, 0, _C8F), (1, 0, _C8F),
          (0, 2, _C0F), (1, 2, _C0F), (0, 3, _C0F), (1, 3, _C0F)],
}
PORDER2 = os.environ.get("PORDER2", "S")
PIECE_ORDER2 = _ORDERS2[PORDER2]
LAST_KEY = tuple(PIECE_ORDER2[-1][:2])
PACE_NUM = int(os.environ.get("PACE_NUM", "1"))
PACE_DEN = int(os.environ.get("PACE_DEN", "1"))
PIPE_DEPTH = int(os.environ.get("PIPE_DEPTH", "3"))
PACE_START = int(os.environ.get("PACE_START", "0"))
PS_BUFS = int(os.environ.get("PS_BUFS", "3"))
E_BUFS = int(os.environ.get("E_BUFS", "10"))
PVA = int(os.environ.get("PVA", "4"))   # PV matmuls emitted mid-step


def build_kernel(repeat: int = 1, loop_n: int = 1):
    nc = bacc.Bacc()
    xqT = nc.dram_tensor("xqT", [D, S], BF16, kind="ExternalInput")
    xkT = nc.dram_tensor("xkT", [D, S], BF16, kind="ExternalInput")
    xvT = nc.dram_tensor("xvT", [D, S], BF16, kind="ExternalInput")
    wq = nc.dram_tensor("wq", [D, E], BF16, kind="ExternalInput")
    wk = nc.dram_tensor("wk", [D, E], BF16, kind="ExternalInput")
    wv = nc.dram_tensor("wv", [D, E], BF16, kind="ExternalInput")
    bq = nc.dram_tensor("bq", [128, 2], F32, kind="ExternalInput")
    bk = nc.dram_tensor("bk", [128, 2], F32, kind="ExternalInput")
    # per-head-contiguous output: [head, S(permuted p-major per 512-block), 65]
    out = nc.dram_tensor("out", [HPC, S, HD + 1], BF16,
                         kind="ExternalOutput")

    with TileContext(nc) as tc:
        with tc.tile_pool(name="wsb", bufs=1) as wsb, \
             tc.tile_pool(name="xsb", bufs=12) as xsb, \
             tc.tile_pool(name="qkv", bufs=1) as qkv, \
             tc.tile_pool(name="esb", bufs=E_BUFS) as esb, \
             tc.tile_pool(name="osb", bufs=4) as osb, \
             tc.tile_pool(name="psp", bufs=PS_BUFS, space="PSUM") as psp, \
             tc.tile_pool(name="pvp", bufs=2, space="PSUM") as pvp:

            # ---- weights / constants; wk first (feeds the first matmuls) ----
            wk_t = wsb.tile([128, DCH, E], BF16)
            wkr = wk.rearrange("(c p) e -> p c e", p=128)
            nc.sync.dma_start(wk_t[:, 0:4], wkr[:, 0:4])
            # touch Exp early so the ACT table load happens during the DMAs;
            # input is a memset tile so it doesn't wait on any DMA
            warm = wsb.tile([128, 1], F32)
            nc.gpsimd.memset(warm[:], 0.0)
            warm2 = wsb.tile([128, 1], F32)
            nc.scalar.activation(warm2[:], warm[:], AF.Exp)
            # PE p-state warmup: dummy matmuls on memset tiles while the
            # first DMAs are in flight, so the 0.65->2.4GHz clock ramp
            # completes before any real matmul issues.  Sized to end just
            # before the first projection's inputs land (~4.2us).
            NWARM = int(os.environ.get("NWARM", "2"))
            if NWARM:
                dmw = wsb.tile([128, 128], BF16, name="dmw")
                nc.gpsimd.memset(dmw[:], 0.0)
                dmx = wsb.tile([128, 512], BF16, name="dmx")
                nc.gpsimd.memset(dmx[:], 0.0)
                dps = psp.tile([128, 512], F32, tag="ps", name="dps")
                for _ in range(NWARM):
                    nc.tensor.matmul(dps[:], dmw[:], dmx[:],
                                     start=True, stop=True)
                dsink = wsb.tile([128, 1], F32, name="dsink")
                nc.vector.tensor_copy(dsink[:], dps[:, 0:1])

            def load_wq():
                wq_t = wsb.tile([128, DCH, E], BF16, name="wq_t")
                wqr = wq.rearrange("(c p) e -> p c e", p=128)
                nc.sync.dma_start(wq_t[:, 0:4], wqr[:, 0:4])
                nc.sync.dma_start(wq_t[:, 4:8], wqr[:, 4:8])
                bq_t = wsb.tile([128, 2], F32, name="bq_t")
                nc.sync.dma_start(bq_t[:], bq[:])
                return wq_t, bq_t

            def load_wv():
                wv_t = wsb.tile([128, DCH, E], BF16, name="wv_t")
                wvr = wv.rearrange("(c p) e -> p c e", p=128)
                nc.sync.dma_start(wv_t[:, 0:4], wvr[:, 0:4])
                nc.sync.dma_start(wv_t[:, 4:8], wvr[:, 4:8])
                return wv_t

            def load_x(src, si, chunked=False, defer_half=False):
                sl = slice(512 * si, 512 * (si + 1))
                x_t = xsb.tile([128, DCH, 512], BF16, tag="x", name=f"x_{si}")
                xr = src[:, sl].rearrange("(c p) s -> p c s", p=128)
                if defer_half:
                    nc.sync.dma_start(x_t[:, :, 0:256], xr[:, :, 0:256])
                    return x_t, lambda: nc.sync.dma_start(
                        x_t[:, :, 256:512], xr[:, :, 256:512])
                if chunked:
                    nc.sync.dma_start(x_t[:, 0:4], xr[:, 0:4])
                    nc.sync.dma_start(x_t[:, 4:8], xr[:, 4:8])
                else:
                    nc.sync.dma_start(x_t[:], xr)
                return x_t

            import contextlib

            def body_scope():
                if loop_n > 1:
                    return tc.For_i(0, loop_n, 1)
                return contextlib.nullcontext()

            for _ in range(repeat):
              with body_scope():
                # persistent per-iteration products
                QT_t = qkv.tile([128, 2, S], BF16, tag="QT", name="QT_t")
                KT_t = qkv.tile([128, 2, S], BF16, tag="KT", name="KT_t")
                V_t = qkv.tile([128, SKC, EV], BF16, tag="V", name="V_t")

                # ---- projection groups (~1-1.7us of PE work each) ----
                def proj_qk_group(x_t, w_t, b_t, o_t, si, et, kt):
                    """One et-group of a Q/K projection for s-block si."""
                    sl = slice(512 * si, 512 * (si + 1))
                    ps = psp.tile([128, 512], F32, tag="ps", name="ps_qk")
                    for c in range(DCH):
                        nc.tensor.matmul(
                            ps[:], w_t[:, c, 128 * et:128 * (et + 1)],
                            x_t[:, c], start=(c == 0), stop=(c == DCH - 1))
                    if BIAS_ENG == "A":
                        nc.scalar.add(o_t[:, et, sl], ps[:],
                                      b_t[:, et:et + 1])
                    else:
                        nc.vector.tensor_scalar(
                            out=o_t[:, et, sl], in0=ps[:],
                            scalar1=b_t[:, et:et + 1], scalar2=None,
                            op0=ALU.add)

                def proj_v_group(x_t, si, k):
                    """One sk-chunk (128 rows) of the V projection."""
                    psv = psp.tile([128, 512], F32, tag="ps", name="ps_v")
                    for c in range(DCH):
                        nc.tensor.matmul(
                            psv[:, 0:E], x_t[:, c, 128 * k:128 * (k + 1)],
                            wv_t[:, c], start=(c == 0), stop=(c == DCH - 1))
                    # strided copy that skips each head's ones column
                    dst = V_t[:, 4 * si + k, :].rearrange(
                        "(h e) -> h e", h=HPC)[:, :, 0:HD]
                    src = psv[:, 0:E].rearrange("(h e) -> h e", h=HPC)
                    if VCOPY_ENG == "A":
                        nc.scalar.copy(dst, src)
                    else:
                        nc.vector.tensor_copy(dst, src)

                # set the V ones columns once (no input deps -> runs
                # during the initial DMA wait)
                nc.gpsimd.memset(
                    V_t.rearrange("k (h e) -> k h e", h=HPC)[:, :, :, HD], 1.0)

                # ---- filler queue machinery ----
                x_state = {}

                def ensure_x(kind, si):
                    key = (kind, si)
                    if key not in x_state:
                        src = {"k": xkT, "q": xqT, "v": xvT}[kind]
                        x_state[key] = load_x(src, si, chunked=(key in (("k", 0), ("q", 0), ("k", 1), ("q", 1), ("k", 2), ("v", 2), ("k", 3), ("q", 2), ("v", 3))))
                    return x_state[key]

                emitted = set()
                state = {"cpl": 0, "fill": 0}
                queue = []

                def emit_group(tag):
                    if tag in emitted:
                        return
                    emitted.add(tag)
                    kind = tag[0]
                    if kind == "K":
                        _, si, et = tag
                        proj_qk_group(ensure_x("k", si), wk_t, bk_t, KT_t,
                                      si, et, kt=True)
                    elif kind == "Q":
                        _, sqt, pr = tag
                        proj_qk_group(ensure_x("q", sqt), wq_t, bq_t, QT_t,
                                      sqt, pr, kt=False)
                    else:
                        _, si, k = tag
                        proj_v_group(ensure_x("v", si), si, k)
                    state["fill"] += 1

                def need(*tags):
                    for t in tags:
                        emit_group(t)

                def pace_fill():
                    state["cpl"] += 1
                    while queue and queue[0] in emitted:
                        queue.pop(0)
                    c_eff = state["cpl"] - PACE_START
                    if (queue and c_eff > 0
                            and state["fill"] * PACE_DEN <= c_eff * PACE_NUM):
                        emit_group(queue.pop(0))

                ovs = {}
                # cross-piece PV pipeline: entries [emit_part_fn, stage, fin]
                gq = []

                def gq_step(force=False, depth=PIPE_DEPTH):
                    """Emit the next PV chunk if the pipeline lag is exceeded."""
                    if not gq:
                        return
                    if not force and len(gq) <= depth:
                        return
                    ent = gq[0]
                    ent[0](ent[1])
                    ent[1] += 1
                    if ent[1] == 2:
                        gq.pop(0)
                        if ent[2] is not None:
                            ent[2]()

                def exp_full(st, eng):
                    """exp of one [128,1024] score tile on ACT or DVE."""
                    e_t = esb.tile([128, 1024], BF16, tag="e", name="e_t")
                    if eng == "A":
                        nc.scalar.activation(e_t[:], st[:], AF.Exp,
                                             scale=float(SCALE))
                    else:
                        ti = e_t.bitcast(I16)
                        nc.vector.tensor_scalar(
                            out=ti[:], in0=st[:], scalar1=FE_A,
                            scalar2=FE_B, op0=ALU.mult, op1=ALU.add)
                    return e_t

                def attention_piece(pr, sqt, cklist, first, final,
                                    last=False, pidx=0):
                    sq0 = 512 * sqt
                    ncpl = len(cklist) // 2
                    need(("Q", sqt, pr))
                    pvh = [pvp.tile([128, 4, 80], F32, tag="pv", bufs=2,
                                    name=f"pv{h}") for h in range(2)]

                    def emit_pv_part(ets, cks, cpl, part):
                        """part 0: h0/q0; part 1: rest; parts 10/11/12 stage
                        the final entry as h0-q1 / h1-q0+q1 for tail overlap."""
                        if part in (0, 10):
                            for ck in cks:
                                need(("V", ck // 4, ck % 4))
                        if PVA == 8:
                            p0, p1 = [(0, 0), (0, 1)], [(1, 0), (1, 1)]
                        else:
                            p0, p1 = [(0, 0)], [(0, 1), (1, 0), (1, 1)]
                        plan = {0: p0, 1: p1,
                                10: [(0, 0)], 11: [(0, 1)],
                                12: [(1, 0), (1, 1)]}[part]
                        for h, q in plan:
                            hh = 2 * pr + h
                            e_t = ets[h]
                            for j in range(4):
                                nc.tensor.matmul(
                                    pvh[h][:, j, 0:65],
                                    e_t[:, 512 * q + 128 * j:
                                        512 * q + 128 * (j + 1)],
                                    V_t[:, cks[q], 65 * hh:65 * hh + 65],
                                    start=(cpl == 0 and q == 0 and j == 0),
                                    stop=(cpl == ncpl - 1 and q == 1
                                          and j == 3))

                    def finalize(hs=(0, 1)):
                        for h in hs:
                            hh = 2 * pr + h
                            # stored (p-major, j-minor) within the 512-block
                            # so the DMA sees 1040B contiguous runs; host
                            # unpermutes (s = j*128 + p).
                            dst = out[hh, sq0:sq0 + 512, :]
                            dst = dst.rearrange("(p j) e -> p j e", p=128)
                            if first and final:
                                ov = osb.tile([128, 4, 65], F32, tag="ov",
                                              bufs=6, name=f"ov{pr}{sqt}{h}")
                                nc.vector.tensor_copy(ov[:],
                                                      pvh[h][:, :, 0:65])
                                nc.sync.dma_start(dst, ov[:])
                            elif first:
                                ov = osb.tile([128, 4, 65], F32, tag="ov",
                                              bufs=6, name=f"ov{pr}{sqt}{h}")
                                ovs[(pr, sqt, h)] = ov
                                nc.vector.tensor_copy(ov[:],
                                                      pvh[h][:, :, 0:65])
                            elif not final:
                                ov = ovs[(pr, sqt, h)]
                                nc.vector.tensor_tensor(
                                    out=ov[:], in0=ov[:],
                                    in1=pvh[h][:, :, 0:65], op=ALU.add)
                            else:
                                ov = ovs.pop((pr, sqt, h))
                                ovb = osb.tile([128, 4, 65], BF16, tag="ov",
                                               bufs=14, name=f"ovb{pr}{sqt}{h}")
                                nc.vector.tensor_tensor(
                                    out=ovb[:], in0=ov[:],
                                    in1=pvh[h][:, :, 0:65], op=ALU.add)
                                nc.sync.dma_start(dst, ovb[:])

                    depth = 1 if last else PIPE_DEPTH
                    for cpl in range(ncpl):
                        cks = (cklist[2 * cpl], cklist[2 * cpl + 1])
                        for ck in cks:
                            need(("K", ck // 4, pr))
                        if last and cpl == ncpl - 1:
                            pat = FE_PAT_LAST
                        elif cpl % 2 == 0:
                            pat = FE_PAT
                        else:
                            pat = FE_PAT2
                        ets = []
                        for h in range(2):
                            hh = 2 * pr + h
                            st = psp.tile([128, 1024], F32, tag="ps",
                                          name="st")
                            hsl = slice(64 * h, 64 * h + 64)
                            for q in range(2):
                                nc.tensor.matmul(
                                    st[:, 512 * q:512 * (q + 1)],
                                    KT_t[hsl, pr,
                                         128 * cks[q]:128 * (cks[q] + 1)],
                                    QT_t[hsl, pr, sq0:sq0 + 512],
                                    start=True, stop=True)
                            ets.append(exp_full(st, pat[h]))
                            if h == 0:
                                # PV chunk between the h-pairs hides the
                                # st-ring recycle latency
                                gq_step(depth=depth)
                        pace_fill()
                        gq.append([
                            lambda p, e=ets, c=cks, pp=cpl:
                                emit_pv_part(e, c, pp, p),
                            0,
                            finalize if cpl == ncpl - 1 else None])
                        gq_step(depth=depth)
                    if last:
                        # staged drain of the final entry: h0's copy+store
                        # chain starts while h1's last PV matmuls still run
                        while len(gq) > 1:
                            gq_step(force=True)
                        fn, stage, fin = gq.pop(0)
                        if stage == 0:
                            fn(10)
                        fn(11)
                        fin(hs=(0,))
                        fn(12)
                        fin(hs=(1,))

                # ---- bootstrap: si=0/1 projections direct; DMA order is
                # wk[0:4] -> xk0[0:4] -> wk[4:8] -> xk0[4:8] -> bk so the
                # first projection group's matmuls start as early as possible
                xk0 = xsb.tile([128, DCH, 512], BF16, tag="x", name="x_0")
                xk0r = xkT[:, 0:512].rearrange("(c p) s -> p c s", p=128)
                nc.sync.dma_start(xk0[:, 0:2], xk0r[:, 0:2])
                nc.sync.dma_start(xk0[:, 2:4], xk0r[:, 2:4])
                nc.sync.dma_start(wk_t[:, 4:8], wkr[:, 4:8])
                nc.sync.dma_start(xk0[:, 4:6], xk0r[:, 4:6])
                nc.sync.dma_start(xk0[:, 6:8], xk0r[:, 6:8])
                x_state[("k", 0)] = xk0
                bk_t = wsb.tile([128, 2], F32)
                nc.sync.dma_start(bk_t[:], bk[:])
                need(("K", 0, 0), ("K", 0, 1))
                wq_t, bq_t = load_wq()
                need(("Q", 0, 0), ("Q", 0, 1))
                need(("K", 1, 0), ("K", 1, 1))
                wv_t = load_wv()
                xv0, fin0 = load_x(xvT, 0, defer_half=True)
                xv1, fin1 = load_x(xvT, 1, defer_half=True)
                x_state[("v", 0)] = xv0
                x_state[("v", 1)] = xv1
                need(("V", 0, 0), ("V", 0, 1))
                need(("V", 1, 0), ("V", 1, 1))
                fin0()
                fin1()
                need(("V", 0, 2), ("V", 0, 3))
                need(("V", 1, 2), ("V", 1, 3))
                if PORDER2 == "S":
                    prefetch = (("q", 1), ("k", 2), ("v", 2), ("k", 3),
                                ("v", 3), ("q", 2), ("q", 3))
                else:
                    prefetch = (("k", 2), ("v", 2), ("k", 3), ("v", 3),
                                ("q", 1), ("q", 2), ("q", 3))
                for _k, _si in prefetch:
                    ensure_x(_k, _si)
                state["fill"] = 0  # bootstrap groups don't count against pacing

                # everything else rides the paced filler queue
                if PORDER2 == "S":
                    queue.extend([
                        ("Q", 1, 0), ("Q", 1, 1),
                        ("K", 2, 0), ("K", 2, 1),
                        ("V", 2, 0), ("V", 2, 1), ("V", 2, 2), ("V", 2, 3),
                        ("K", 3, 0), ("K", 3, 1),
                        ("V", 3, 0), ("V", 3, 1), ("V", 3, 2), ("V", 3, 3),
                        ("Q", 2, 0), ("Q", 2, 1),
                        ("Q", 3, 0), ("Q", 3, 1),
                    ])
                elif PORDER2 == "F4":
                    queue.extend([
                        ("Q", 1, 0), ("Q", 1, 1),
                        ("K", 2, 0), ("K", 3, 0), ("K", 2, 1), ("K", 3, 1),
                        ("V", 2, 0), ("V", 2, 1), ("V", 2, 2), ("V", 2, 3),
                        ("V", 3, 0), ("V", 3, 1), ("V", 3, 2), ("V", 3, 3),
                        ("Q", 2, 0), ("Q", 2, 1),
                        ("Q", 3, 0), ("Q", 3, 1),
                    ])
                else:
                    queue.extend([
                        ("K", 2, 0), ("K", 3, 0), ("K", 2, 1), ("K", 3, 1),
                        ("V", 2, 0), ("V", 2, 1), ("V", 2, 2), ("V", 2, 3),
                        ("V", 3, 0), ("V", 3, 1), ("V", 3, 2), ("V", 3, 3),
                        ("Q", 1, 0), ("Q", 1, 1),
                        ("Q", 2, 0), ("Q", 2, 1),
                        ("Q", 3, 0), ("Q", 3, 1),
                    ])

                seen = {}
                total = {}
                for (pr, sqt, cklist) in PIECE_ORDER2:
                    total[(pr, sqt)] = total.get((pr, sqt), 0) + len(cklist)
                for i, (pr, sqt, cklist) in enumerate(PIECE_ORDER2):
                    first = (pr, sqt) not in seen
                    seen[(pr, sqt)] = seen.get((pr, sqt), 0) + len(cklist)
                    final = seen[(pr, sqt)] == total[(pr, sqt)]
                    attention_piece(pr, sqt, cklist, first, final,
                                    last=(i == len(PIECE_ORDER2) - 1),
                                    pidx=i)
    nc.compile()
    return nc


_NC_CACHE = {}


def _get_nc(repeat: int = 1, loop_n: int = 1):
    key = (repeat, loop_n)
    if key not in _NC_CACHE:
        _NC_CACHE[key] = build_kernel(repeat, loop_n)
    return _NC_CACHE[key]


def _shard_inputs(q, k, v, Wq, bq, Wk, bk, Wv, bv):
    """Build the 8 per-core input maps (host-side marshaling)."""
    xT = {}
    for b in range(B):
        xT[("q", b)] = np.ascontiguousarray(np.asarray(q)[b].T).astype(BF16NP)
        xT[("k", b)] = np.ascontiguousarray(np.asarray(k)[b].T).astype(BF16NP)
        xT[("v", b)] = np.ascontiguousarray(np.asarray(v)[b].T).astype(BF16NP)
    Wq, Wk, Wv = (np.asarray(a, np.float32) for a in (Wq, Wk, Wv))
    bq, bk = (np.asarray(a, np.float32) for a in (bq, bk))
    in_maps = []
    for c in range(NCORES):
        b, g = divmod(c, HPC)
        sl = slice(E * g, E * (g + 1))
        in_maps.append({
            "xqT": xT[("q", b)], "xkT": xT[("k", b)], "xvT": xT[("v", b)],
            "wq": np.ascontiguousarray(Wq[:, sl]).astype(BF16NP),
            "wk": np.ascontiguousarray(Wk[:, sl]).astype(BF16NP),
            "wv": np.ascontiguousarray(Wv[:, sl]).astype(BF16NP),
            "bq": np.ascontiguousarray(bq[sl].reshape(2, 128).T),
            "bk": np.ascontiguousarray(bk[sl].reshape(2, 128).T),
        })
    return in_maps


def kernel(q, k, v, Wq, bq, Wk, bk, Wv, bv):
    nc = _get_nc()
    in_maps = _shard_inputs(q, k, v, Wq, bq, Wk, bk, Wv, bv)
    res = run_bass_kernel_spmd(nc, in_maps, core_ids=list(range(NCORES)))
    bv = np.asarray(bv, np.float32)
    outp = np.empty((B, S, D), np.float32)
    for c in range(NCORES):
        b, g = divmod(c, HPC)
        o = res.results[c]["out"]  # [4, S(permuted), 65]: out cols + denom
        for h in range(HPC):
            # device stored 512-blocks in (p-major, j-minor) row order;
            # true s = block*512 + j*128 + p, stored row = block*512 + p*4 + j
            blk = o[h].astype(np.float32)
            blk = blk.reshape(4, 128, 4, 65).transpose(0, 2, 1, 3)
            blk = blk.reshape(S, 65)
            c0 = E * g + HD * h
            outp[b, :, c0:c0 + HD] = (blk[:, :HD] / blk[:, HD:HD + 1]
                                      + bv[c0:c0 + HD])
    return outp
